# revision 1
# baseline (speedup 1.0000x reference)
"""BIDE forward kernel for Trainium2, 8-core data parallel over B — v3.

Two parallel pipelines per batch row (2 rows per core):

logZ path (enumerates all 2^16 patterns as a 256x256 table):
  table[hi, lo] = sum_h cos(zhi)cos(zlo) - sin(zhi)sin(zlo)  (two K=128
  matmuls over trig tables), logZ = 60 + ln(sum exp(table - 60)).

logit_x path (direct, no gather): q[h, t] = sum_n W'[h,n] bit_n(x_t) + r'
  as a K=17 matmul over host-precomputed bit-planes of x, then
  logit_x[t] = sum_h cos(2*pi*q[h,t]) via Sin + one-hot-column matmuls
  that fold h into a [16, 512] PSUM tile already shaped like the output.

Range reduction (Sin only accepts [-pi, pi]; DVE has no mod op and
rint+subtract costs two DVE passes): the matmul's constant row adds
cp + 192 so PSUM holds q' = q + cp + 192 in [128, 256), where bf16's ULP
is exactly 1 — a single DVE f32->bf16 copy rounds q' to the nearest
INTEGER. A second PE matmul accumulates -I @ round(q') into the same
PSUM bank, leaving w = centered-frac(q + cp), and Sin(2*pi*w) needs no
bias. One DVE pass + free PE work instead of two DVE passes.

Per-element gathers are unusable on this part: indirect-DMA descriptors
retire at ~4.5ns each *serialized* (measured: completion semaphores lag
data by 20-30us for 8k descriptors), and gpsimd ap_gather ucode runs
~27us per 512 indices. Hence the direct logit_x computation.
"""

import numpy as np
import ml_dtypes
from contextlib import ExitStack

import concourse.bacc as bacc
import concourse.bass as bass
from concourse import mybir
from concourse.bass_utils import run_bass_kernel_spmd
from concourse.tile import TileContext

F32 = mybir.dt.float32
BF16 = mybir.dt.bfloat16
I32 = mybir.dt.int32

TWO_PI = float(np.float32(2.0 * np.pi))
INV_2PI = 1.0 / (2.0 * np.pi)
# logits peak ~89: shift exp so it stays in fp32 / the ACT Ln spline range
EXP_SHIFT = 60.0
# q + MAGIC lands in [128, 256) where bf16 ULP = 1, so bf16-rounding = rint
MAGIC = 192.0

N_CORES = 8
B, H, T = 16, 128, 4096
BPC = B // N_CORES  # batch rows per core (2)


def _build():
    nc = bacc.Bacc("TRN2", target_bir_lowering=False, debug=False)

    # packed weights, one DMA: cols [0:1024) wp (table z weights, quarter
    # g of row b at 128*(4b+g); rows 0-7 W'bits, 8 r'+cp, 9 MAGIC),
    # [1024:1280) bit-plane enumeration, [1280:1536) wx (x-path z weights;
    # rows 0-15 W'.T, 16 r'+0.25, 17 MAGIC — MAGIC rides its own row since
    # bf16 ULP at 192 is 1.0 and would wipe out r')
    wmeta = nc.dram_tensor("wmeta", [18, 1536], BF16, kind="ExternalInput")
    # bit-planes of x: row n = bit_n(x[b, t]), rows 16-17 = 1; row b at 4096b
    bitsx = nc.dram_tensor("bitsx", [18, 8192], BF16, kind="ExternalInput")
    # packed: cols [0:128) -I, [128:384) h-sum one-hots hsw[h, 16v+m]=(m==v)
    aux = nc.dram_tensor("aux", [128, 384], BF16, kind="ExternalInput")
    # negsel[k, m] = -1 if m//8 == k else 0 (broadcasts -ln S_b)
    negsel_in = nc.dram_tensor("negsel", [2, 16], F32, kind="ExternalInput")
    out = nc.dram_tensor("out", [BPC, T], F32, kind="ExternalOutput")

    with ExitStack() as ctx:
        tc = ctx.enter_context(TileContext(nc))
        sb = ctx.enter_context(tc.tile_pool(name="sb", bufs=1))
        psa = ctx.enter_context(tc.tile_pool(name="psa", bufs=2, space="PSUM"))
        pst = ctx.enter_context(tc.tile_pool(name="pst", bufs=2, space="PSUM"))
        psh = ctx.enter_context(tc.tile_pool(name="psh", bufs=1, space="PSUM"))
        pss = ctx.enter_context(tc.tile_pool(name="pss", bufs=1, space="PSUM"))

        # ---- input loads
        bitsx_sb = sb.tile([18, 8192], BF16, tag="bitsx")
        wmeta_sb = sb.tile([18, 1536], BF16, tag="wmeta")
        aux_sb = sb.tile([128, 384], BF16, tag="aux")
        negsel = sb.tile([2, 16], F32, tag="negsel")
        nc.sync.dma_start(out=wmeta_sb[:], in_=wmeta[:])
        nc.sync.dma_start(out=bitsx_sb[:, 0:4096], in_=bitsx[:, 0:4096])
        nc.sync.dma_start(out=aux_sb[:], in_=aux[:])
        nc.sync.dma_start(out=bitsx_sb[:, 4096:8192], in_=bitsx[:, 4096:8192])
        nc.sync.dma_start(out=negsel[:], in_=negsel_in[:])

        # ---- constants
        ones = sb.tile([128, 1], F32, tag="ones")
        nc.vector.memset(ones[:], 1.0)

        neg_shift = sb.tile([128, 1], F32, tag="neg_shift")
        nc.vector.memset(neg_shift[:], -EXP_SHIFT)
        e_sb = sb.tile([128, 1024], BF16, tag="e")

        sums2 = sb.tile([128, 2], F32, tag="sums2")
        hs_ps = psh.tile([16, 512], F32, tag="hs")
        tb_ps = []

        # q' group is CLOSED (start=T stop=T), the DVE cast legally reads
        # it, then -I @ round(q') re-opens the closed region with
        # start=False and accumulates in place (w = q' - round(q')).
        def q_unit(mk_qmms, tag, regions, sub_on_dve=False, cast_on_act=False):
            qa = psa.tile([128, 1024], F32, tag="qa")
            mk_qmms(qa)
            width = max(end for _, end in regions)
            tq = sb.tile([128, width], BF16, tag=tag)
            if cast_on_act:
                # ACT Copy also rounds f32->bf16 (engine load balance)
                nc.scalar.activation(
                    out=tq[:], in_=qa[:, 0:width],
                    func=mybir.ActivationFunctionType.Copy,
                )
            else:
                nc.vector.tensor_scalar(
                    out=tq[:], in0=qa[:, 0:width], scalar1=0.0, scalar2=None,
                    op0=mybir.AluOpType.add,
                )
            if sub_on_dve:
                # DVE w = q' - round(q') to SBUF (PE/DVE load balance)
                wsb = sb.tile([128, width], BF16, tag=tag + "w")
                nc.vector.tensor_tensor(
                    out=wsb[:], in0=qa[:, 0:width], in1=tq[:],
                    op=mybir.AluOpType.subtract,
                )
                return wsb
            for off, end in regions:
                nc.tensor.matmul(
                    out=qa[:, off:end], lhsT=aux_sb[:, 0:128],
                    rhs=tq[:, off:end], start=False, stop=True,
                )
            return qa

        for b in range(BPC):
            # ---- table path: one unit per quarter (one open accumulation
            # group per PSUM bank at a time — two opens in one bank corrupt)
            tcos = sb.tile([128, 512], BF16, tag=f"tc{b}")
            tsin = sb.tile([128, 512], BF16, tag=f"ts{b}")
            qt = psa.tile([128, 1024], F32, tag="qa")
            for g in range(4):
                nc.tensor.matmul(
                    out=qt[:, 256 * g : 256 * g + 256],
                    lhsT=wmeta_sb[0:10, 128 * (4 * b + g) : 128 * (4 * b + g) + 128],
                    rhs=wmeta_sb[0:10, 1024:1280],
                    start=True, stop=True,
                )
            qit = sb.tile([128, 1024], I32, tag=f"qit{b}")
            nc.vector.tensor_copy(out=qit[:], in_=qt[:])
            wt = sb.tile([128, 1024], BF16, tag=f"wt{b}")
            nc.vector.tensor_tensor(
                out=wt[:], in0=qt[:], in1=qit[:], op=mybir.AluOpType.subtract,
            )
            for g in range(4):
                dst = tcos if g < 2 else tsin
                nc.scalar.activation(
                    out=dst[:, 256 * (g % 2) : 256 * (g % 2) + 256],
                    in_=wt[:, 256 * g : 256 * g + 256],
                    func=mybir.ActivationFunctionType.Sin,
                    scale=TWO_PI if g < 3 else -TWO_PI,
                )
            # table[hi, lo] = cos(zhi)cos(zlo) - sin(zhi)sin(zlo)
            tp = pst.tile([128, 512], F32, tag="tb")
            tb_ps.append(tp)
            for c in range(2):
                cs = slice(256 * c, 256 * c + 256)
                hi_s = slice(256 + 128 * c, 256 + 128 * c + 128)
                nc.tensor.matmul(
                    out=tp[:, cs], lhsT=tcos[:, hi_s], rhs=tcos[:, 0:256],
                    start=True, stop=False,
                )
                nc.tensor.matmul(
                    out=tp[:, cs], lhsT=tsin[:, hi_s], rhs=tsin[:, 0:256],
                    start=False, stop=True,
                )
            # exp early (costs an extra ACT table load but runs logZ's
            # chain in parallel with the x stream instead of in the tail)
            nc.scalar.activation(
                out=e_sb[:, 512 * b : 512 * b + 512], in_=tp[:],
                func=mybir.ActivationFunctionType.Exp,
                bias=neg_shift[:],
                accum_out=sums2[:, b : b + 1],
            )

            # ---- x path: 8 chunks of 512 t's
            sxb = sb.tile([128, 4096], BF16, tag=f"sx{b}")
            for c in range(4):
                col = 4096 * b + 1024 * c

                def x_qmm(dst, col=col):
                    for h2 in range(2):
                        nc.tensor.matmul(
                            out=dst[:, 512 * h2 : 512 * h2 + 512],
                            lhsT=wmeta_sb[:, 1280 + 128 * b : 1280 + 128 * b + 128],
                            rhs=bitsx_sb[:, col + 512 * h2 : col + 512 * h2 + 512],
                            start=True, stop=True,
                        )

                qxb = q_unit(
                    x_qmm, f"tbf{b}{c}", [(0, 512), (512, 1024)],
                    sub_on_dve=True, cast_on_act=(c == 0),
                )
                nc.scalar.activation(
                    out=sxb[:, 1024 * c : 1024 * c + 1024], in_=qxb[:],
                    func=mybir.ActivationFunctionType.Sin, scale=TWO_PI,
                )
                # h-fold: hs[8b + t//512, j] += sum_h cos
                for h2 in range(2):
                    vg = 8 * b + 2 * c + h2
                    nc.tensor.matmul(
                        out=hs_ps[:],
                        lhsT=aux_sb[:, 128 + 16 * vg : 128 + 16 * vg + 16],
                        rhs=sxb[:, 1024 * c + 512 * h2 : 1024 * c + 512 * h2 + 512],
                        start=(vg == 0), stop=(vg == 15),
                    )

        small_ps = pss.tile([16, 1], F32, tag="small")
        nc.tensor.matmul(
            out=small_ps[0:2, 0:1], lhsT=sums2[:], rhs=ones[:], start=True, stop=True
        )
        logz2 = sb.tile([2, 1], F32, tag="logz2")
        nc.scalar.activation(
            out=logz2[:], in_=small_ps[0:2, 0:1],
            func=mybir.ActivationFunctionType.Ln,
        )
        # broadcast -ln(S_b) to the 16 output partitions (reuses the bank)
        nz_ps = small_ps
        nc.tensor.matmul(out=nz_ps[:], lhsT=negsel[:], rhs=logz2[:], start=True, stop=True)
        nz_sb = sb.tile([16, 1], F32, tag="nzsb")
        nc.vector.tensor_scalar(
            out=nz_sb[:], in0=nz_ps[:], scalar1=-EXP_SHIFT, scalar2=None,
            op0=mybir.AluOpType.add,
        )

        # ---- out[b, t] = logit_x - logZ_b
        o_t = sb.tile([16, 512], F32, tag="o")
        nc.vector.tensor_scalar(
            out=o_t[:], in0=hs_ps[:], scalar1=nz_sb[:], scalar2=None,
            op0=mybir.AluOpType.add,
        )
        for b in range(BPC):
            nc.sync.dma_start(
                out=out[b, :].rearrange("(c j) -> c j", c=8),
                in_=o_t[8 * b : 8 * b + 8, :],
            )

    nc.finalize()
    return nc


_NC = None


def _get_nc():
    global _NC
    if _NC is None:
        _NC = _build()
    return _NC


def _make_in_maps(x, W, r):
    x = np.asarray(x, dtype=np.int32)
    W = np.asarray(W, dtype=np.float32)
    r = np.asarray(r, dtype=np.float32)

    v = np.arange(256, dtype=np.int32)
    k8 = np.arange(8, dtype=np.int32)
    bp8 = ((v[None, :] >> k8[:, None]) & 1).astype(np.float32)  # [8, 256]
    bits = np.ones((10, 256), dtype=np.float32)
    bits[0:8] = bp8
    bits = bits.astype(ml_dtypes.bfloat16)

    k16 = np.arange(16, dtype=np.int32)
    aux = np.zeros((128, 384), dtype=np.float32)
    aux[:, 0:128] = -np.eye(128, dtype=np.float32)
    for vg in range(16):
        aux[:, 128 + 16 * vg + vg] = 1.0
    aux = aux.astype(ml_dtypes.bfloat16)
    negsel = np.zeros((2, 16), dtype=np.float32)
    negsel[0, 0:8] = -1.0
    negsel[1, 8:16] = -1.0

    in_maps = []
    for core in range(N_CORES):
        wmeta = np.zeros((18, 1536), dtype=ml_dtypes.bfloat16)
        wp = wmeta[0:10, 0:1024]
        wmeta[0:10, 1024:1280] = bits
        wxm = wmeta[:, 1280:1536]
        bxs = []
        for b_loc in range(BPC):
            b = BPC * core + b_loc
            Wp = (W[b].T * INV_2PI).astype(ml_dtypes.bfloat16)  # [16, 128]
            rp = (r[b] * INV_2PI).astype(ml_dtypes.bfloat16).astype(np.float32)
            for g in range(4):
                # g: 0 coslo, 1 coshi, 2 sinlo, 3 sinhi
                half = 1 if g in (1, 3) else 0
                cp = 0.25 if g in (0, 1) else 0.0
                cs = slice(128 * (4 * b_loc + g), 128 * (4 * b_loc + g) + 128)
                wp[0:8, cs] = Wp[8 * half : 8 * half + 8]
                # r'+cp stays small (bf16-safe); MAGIC exact in its own row
                # (bf16 ULP at 192 is 1.0 -- adding r'+cp here would erase it)
                wp[8, cs] = ((rp if half else 0.0) + np.float32(cp)).astype(
                    ml_dtypes.bfloat16
                )
                wp[9, cs] = np.float32(MAGIC)
            xs = slice(128 * b_loc, 128 * b_loc + 128)
            wxm[0:16, xs] = Wp
            wxm[16, xs] = (rp + np.float32(0.25)).astype(ml_dtypes.bfloat16)
            wxm[17, xs] = np.float32(MAGIC)
            bx = np.ones((18, 4096), dtype=np.float32)
            bx[0:16] = ((x[b][None, :] >> k16[:, None]) & 1).astype(np.float32)
            bxs.append(bx.astype(ml_dtypes.bfloat16))
        in_maps.append(
            {
                "wmeta": wmeta,
                "bitsx": np.concatenate(bxs, axis=1),
                "aux": aux,
                "negsel": negsel,
            }
        )
    return in_maps


def _run(x, W, r, trace=False):
    nc = _get_nc()
    in_maps = _make_in_maps(x, W, r)
    res = run_bass_kernel_spmd(nc, in_maps, core_ids=list(range(N_CORES)), trace=trace)
    out = np.concatenate([res.results[c]["out"] for c in range(N_CORES)], axis=0)
    return out.astype(np.float32), res


def kernel(x, W, r):
    out, _ = _run(x, W, r)
    return out


def kernel_traced(x, W, r):
    out, res = _run(x, W, r, trace=True)
    return out, res

